# revision 1
# baseline (speedup 1.0000x reference)
"""NT-Xent contrastive loss on 8 Trainium2 NeuronCores — v2.

Strategy (symmetric, fp8-DoubleRow):
  z = concat(z_i, z_j) -> [8192, 256].  Host normalizes (fp64) and
  quantizes zn to fp8e4 (the dot products then run at 0.5 cyc/row via
  MatmulPerfMode.DoubleRow: both 128-K-tiles in one PE pass).

  The 8192x8192 similarity matrix is symmetric, so only ~half is
  computed.  At 512-row granularity (16 half-rows), core c owns
  half-rows {2c, 2c+1}; for each of its half-rows a it computes the
  [512, 512] blocks (a, a+d mod 16) for d = 0..8.  d in 1..7 covers
  each unordered half-row pair once (the d in 9..15 blocks are the
  transposes, owned by the partner); d = 8 (antipodal) is computed by
  both sides, each using only its row sums; d = 0 is the diagonal
  block.  Row sums come from ACT accum_out on the exp pass; column
  sums (the transposed-block contribution to the partner's rows) come
  from ones-vector matmuls over the exp'd tiles and are reassembled on
  the host.

  SPMD: every core receives the same layout rotated by 1024c rows, so
  its own rows are always local rows 0..1023.  Only local columns
  0..5119 are touched -> the input is a pre-transposed, pre-quantized
  [128, 2, 5120] fp8 tensor (k-tile dim 2 matches DoubleRow's layout).

  Per (hr, g, m) with g indexing d-triples {0-2, 3-5, 6-8}: 3 DR
  matmuls -> PSUM [128, 3, 512]; one ACT exp instr [128, 1536] with
  accum_out -> row-sum partial + E tile (f16).  Col sums for the d's
  of group g follow immediately after its 4 m-tiles, keeping PE's
  in-order stream from stalling the ACT pipeline (ACT is the
  bottleneck: 24 instrs x ~1.68us = ~40us/core).

  Host: S[global rows] = sum of row partials + reassembled col sums,
  subtract exp(2*||q(zn_r)||^2 - 2) (the quantized self term the
  device actually computed), lse = 2 + log(S), pos = 2*zn_r.zn_partner
  exact in fp64, loss = mean(lse - pos) per half.
"""

import sys

import numpy as np
import ml_dtypes

_REPO = "/opt/trn_rl_repo"
if _REPO not in sys.path:
    sys.path.insert(0, _REPO)

import concourse.bacc as bacc  # noqa: E402
import concourse.mybir as mybir  # noqa: E402
import concourse.tile as tile  # noqa: E402
from concourse import bass_utils  # noqa: E402

N = 4096
D = 256
TWO_N = 2 * N
N_CORES = 8
ROWS_PER_CORE = TWO_N // N_CORES  # 1024
HB = 512  # half-block granularity
COLS_LOC = 10 * HB  # local columns touched: hr+d in 0..9
EPS = 1e-8

F32 = mybir.dt.float32
F16 = mybir.dt.float16
F8 = mybir.dt.float8e4
NP_F8 = ml_dtypes.float8_e4m3

_cache: dict = {}


def _build(reps: int = 1):
    if reps in _cache:
        return _cache[reps]

    nc = bacc.Bacc("TRN2", target_bir_lowering=False, debug=False)
    zq_dram = nc.dram_tensor("zq", [128, 2, COLS_LOC], F8, kind="ExternalInput")
    # rs[p, 4*hr+m, g]: row-sum partial over d-triple g for local row
    # 512*hr + 128*m + p
    rs_dram = nc.dram_tensor("rs", [128, 8, 3], F32, kind="ExternalOutput")
    # cs[0, 7*hr+(d-1), j]: col sums of block (hr, d), local col 512*(hr+d)+j
    cs_dram = nc.dram_tensor("cs", [1, 14, HB], F32, kind="ExternalOutput")

    mult = mybir.AluOpType.mult
    DR = mybir.MatmulPerfMode.DoubleRow

    with tile.TileContext(nc) as tc:
        with (
            tc.tile_pool(name="const", bufs=1) as pconst,
            tc.tile_pool(name="zin", bufs=2) as pzin,
            tc.tile_pool(name="ep", bufs=10) as pep,
            tc.tile_pool(name="outp", bufs=2) as pout,
            tc.tile_pool(name="psp", bufs=2, space="PSUM") as ppsum,
            tc.tile_pool(name="pcol", bufs=2, space="PSUM") as ppcol,
        ):
            bias_m2 = pconst.tile([128, 1], F32)
            nc.gpsimd.memset(bias_m2, -2.0)
            ones = pconst.tile([128, 1], F16)
            nc.gpsimd.memset(ones, 1.0)

            for _rep in range(reps):
                zq = pzin.tile([128, 2, COLS_LOC], F8, name="zq")
                for j in range(4):
                    c0, c1 = j * 1280, (j + 1) * 1280
                    nc.sync.dma_start(zq[:, :, c0:c1], zq_dram[:, :, c0:c1])

                rs = pout.tile([128, 8, 3], F32, name="rs")
                cs = pout.tile([1, 14, HB], F32, name="cs")

                for hr in range(2):
                    etiles = {}
                    # g descending: the last group's col pass (d=1,2 only)
                    # is the cheapest, shortening the post-ACT tail
                    for g in (2, 1, 0):
                        for m in range(4):
                            st = zq[:, :, 512 * hr + 128 * m : 512 * hr + 128 * m + 128]
                            ps = ppsum.tile([128, 3, HB], F32, name="ps")
                            # one matmul per 512-wide d-window: matmul
                            # output cannot cross a PSUM bank boundary
                            for j in range(3):
                                d = 3 * g + j
                                c0 = 512 * (hr + d)
                                nc.tensor.matmul(
                                    ps[:, j, :],
                                    st,
                                    zq[:, :, c0 : c0 + HB],
                                    start=True,
                                    stop=True,
                                    perf_mode=DR,
                                )
                            et = pep.tile([128, 3, HB], F16, name="et")
                            nc.scalar.activation(
                                et.rearrange("p a b -> p (a b)"),
                                ps.rearrange("p a b -> p (a b)"),
                                mybir.ActivationFunctionType.Exp,
                                bias=bias_m2,
                                scale=2.0,
                                accum_out=rs[:, 4 * hr + m, g : g + 1],
                            )
                            etiles[m] = et
                        # col sums for the d's of this triple (skip d=0, d=8)
                        for j in range(3):
                            d = 3 * g + j
                            if d == 0 or d == 8:
                                continue
                            cp = ppcol.tile([128, HB], F32, name="cp")
                            for m in range(4):
                                nc.tensor.matmul(
                                    cp[0:1, :],
                                    ones,
                                    etiles[m][:, j, :],
                                    start=(m == 0),
                                    stop=(m == 3),
                                )
                            nc.vector.tensor_scalar(
                                cs[0:1, 7 * hr + d - 1, :],
                                cp[0:1, :],
                                1.0,
                                None,
                                mult,
                            )

                nc.gpsimd.dma_start(rs_dram[:], rs.rearrange("p a b -> p (a b)"))
                nc.gpsimd.dma_start(cs_dram[:], cs.rearrange("p a b -> p (a b)"))

    nc.compile()
    _cache[reps] = nc
    return nc


def _prep_inputs(z: np.ndarray):
    """z: [8192, 256] fp32.  Returns (zn fp64, znq fp64-of-fp8, in_maps)."""
    z64 = z.astype(np.float64)
    nrm = np.sqrt((z64 * z64).sum(1))
    zn = z64 / np.maximum(nrm, EPS)[:, None]
    znq8 = zn.astype(np.float32).astype(NP_F8)
    znq = znq8.astype(np.float64)
    in_maps = []
    for c in range(N_CORES):
        rolled = np.roll(znq8, -ROWS_PER_CORE * c, axis=0)[:COLS_LOC]  # [5120, 256]
        # zq[p, i, r] = znq[(r + 1024c) % 8192, 128*i + p]
        zt = np.ascontiguousarray(rolled.reshape(COLS_LOC, 2, 128).transpose(2, 1, 0))
        in_maps.append({"zq": zt})
    return zn, znq, in_maps


def _run_device(in_maps, trace: bool = False):
    nc = _build()
    res = bass_utils.run_bass_kernel_spmd(
        nc, in_maps, core_ids=list(range(N_CORES)), trace=trace
    )
    S = np.zeros(TWO_N, np.float64)
    for c in range(N_CORES):
        rs = np.asarray(res.results[c]["rs"]).astype(np.float64)  # [128, 8, 3]
        cs = np.asarray(res.results[c]["cs"]).astype(np.float64).reshape(14, HB)
        base = ROWS_PER_CORE * c
        # row partials: local row 512*hr + 128*m + p, global (base + local) % 2N
        loc_rows = rs.sum(-1)  # [128, 8] over g
        for hrm in range(8):
            hr, m = divmod(hrm, 4)
            r0 = base + 512 * hr + 128 * m
            idx = (r0 + np.arange(128)) % TWO_N
            S[idx] += loc_rows[:, hrm]
        # col sums: block (hr, d): local cols 512*(hr+d) + j
        for hr in range(2):
            for d in range(1, 8):
                r0 = base + 512 * (hr + d)
                idx = (r0 + np.arange(HB)) % TWO_N
                S[idx] += cs[7 * hr + d - 1]
    return S, res.exec_time_ns


def _finalize(zn: np.ndarray, znq: np.ndarray, S: np.ndarray):
    selfsim_q = (znq * znq).sum(1)
    masked = S - np.exp(2.0 * selfsim_q - 2.0)
    lse = 2.0 + np.log(masked)
    pos = 2.0 * (zn * np.roll(zn, -N, axis=0)).sum(1)
    term = lse - pos
    loss_i = term[:N].sum() / N
    loss_j = term[N:].sum() / N
    return np.float32(loss_i), np.float32(loss_j)


def kernel(**inputs) -> np.ndarray:
    z_i = np.asarray(inputs["z_i"], dtype=np.float32)
    z_j = np.asarray(inputs["z_j"], dtype=np.float32)
    z = np.concatenate([z_i, z_j], axis=0)
    zn, znq, in_maps = _prep_inputs(z)
    S, _ = _run_device(in_maps, trace=False)
    return _finalize(zn, znq, S)



# revision 17
# speedup vs baseline: 3.8087x; 3.8087x over previous
"""NT-Xent contrastive loss on 8 Trainium2 NeuronCores — v3.

Strategy (symmetric half-matrix, fp8-DoubleRow mains, ACT+DVE exp split):
  z = concat(z_i, z_j) -> [8192, 256].  Host normalizes (fp64), scales by
  sqrt(ALPHA) with ALPHA = 2048/ln2, and quantizes to fp8e4 so PE's
  DoubleRow matmuls produce PSUM = ALPHA * s directly (s = cosine sim of
  the quantized embeddings).

  The 8192x8192 similarity matrix is symmetric: at 512-row granularity
  core c owns half-rows {2c, 2c+1}; for each its half-row it computes the
  [512, 512] blocks (a, a+d mod 16), d = 0..8.  d in 1..7 covers each
  unordered pair once (row sums for a, col sums for a+d); d = 8 is
  computed by both partners (row sums only); d = 0 is the diagonal.

  exp work is split across two engines to beat the single-engine roofline:
    * ACT triples: one activation instr per (hr,g,m) over [128, 3*512]
      PSUM: exp(PSUM * 2/ALPHA - 2) -> f16 et tile + accum_out row sums.
    * DVE triples: Schraudolph exp — i16 = trunc(PSUM + B_DVE) and the
      i16 BITS, read as fp16, equal exp(2s-2) with a mean-one sawtooth
      error (B_DVE is numerically calibrated so E[approx/exact] = 1).
      One tensor_scalar (f32 psum -> i16 sbuf) + one 2x-mode f16
      tensor_reduce for row-sum partials.
  Assignment: triple index T = 12*hr + 4*g + m; DVE iff T%8 in {2,5,7}
  (9 of 24), balancing ACT ~25us vs DVE ~24us busy per rep.

  Col sums (the transposed-block contribution to the partner's rows) are
  ones-vector matmuls over the et tiles, accumulated per (hr,g) into
  partition row j of a single PSUM bank, DMA'd straight from PSUM by
  gpsimd SWDGE.  PE emits the col matmuls for triple i-2 after the main
  matmuls of triple i, so PE never stalls the ACT/DVE pipeline.

  SPMD: every core receives the same layout rotated by 1024c rows; input
  is a pre-transposed pre-quantized [128, 2, 5120] fp8 tensor.

  Host: S[global rows] = row partials (f32 ACT accum + f16 DVE partials)
  + reassembled col sums, minus exp(2*||q(zn_r)||^2 - 2), lse = 2 +
  log(S), pos exact in fp64, loss = mean(lse - pos) per half.
"""

import sys

import numpy as np
import ml_dtypes

_REPO = "/opt/trn_rl_repo"
if _REPO not in sys.path:
    sys.path.insert(0, _REPO)

import concourse.bacc as bacc  # noqa: E402
import concourse.mybir as mybir  # noqa: E402
import concourse.tile as tile  # noqa: E402
from concourse import bass_utils  # noqa: E402

N = 4096
D = 256
TWO_N = 2 * N
N_CORES = 8
ROWS_PER_CORE = TWO_N // N_CORES  # 1024
HB = 512  # half-block granularity
COLS_LOC = 10 * HB  # local columns touched: hr+d in 0..9
EPS = 1e-8

ALPHA = 2048.0 / np.log(2.0)  # PSUM = ALPHA * s
SQA = float(np.sqrt(ALPHA))
B_DVE = 12360.2374  # calibrated: E[schraudolph/exp] = 1 under truncation

F32 = mybir.dt.float32
F16 = mybir.dt.float16
I16 = mybir.dt.int16
F8 = mybir.dt.float8e4
NP_F8 = ml_dtypes.float8_e4m3

# triple schedule: T = 12*hr + 4*g + m, processed in T order
TRIPLES = [(hr, g, m) for hr in range(2) for g in range(3) for m in range(4)]
DVE_SET = frozenset(t for t in range(24) if t % 3 == 1)
DVE_SLOT = {t: i for i, t in enumerate(sorted(DVE_SET))}
N_DVE = len(DVE_SET)  # 8
ACT_SLOT = {t: i for i, t in enumerate(t for t in range(24) if t not in DVE_SET)}
N_ACT = len(ACT_SLOT)  # 16

_cache: dict = {}


def _build(reps: int = 1):
    if reps in _cache:
        return _cache[reps]

    nc = bacc.Bacc("TRN2", target_bir_lowering=False, debug=False)
    zq_dram = nc.dram_tensor("zq", [128, 2, COLS_LOC], F8, kind="ExternalInput")
    # rs[p, slot]: ACT row-sum partial (sum over the 3 d's of the triple's
    # g) for local row 512*hr + 128*m + p, slot = ACT_SLOT[T]
    rs_dram = nc.dram_tensor("rs", [128, N_ACT], F32, kind="ExternalOutput")
    # rs16[p, slot, j]: DVE row-sum partial for d = 3*g+j of DVE triple slot
    rs16_dram = nc.dram_tensor("rs16", [128, N_DVE, 3], F16, kind="ExternalOutput")
    # cs[hr, g, j, :]: col sums of block (hr, d=3g+j), local col 512*(hr+d)+c
    cs_dram = nc.dram_tensor("cs", [2, 3, 3, HB], F32, kind="ExternalOutput")

    DR = mybir.MatmulPerfMode.DoubleRow
    ADD = mybir.AluOpType.add
    EXP = mybir.ActivationFunctionType.Exp
    AXX = mybir.AxisListType.X

    with tile.TileContext(nc) as tc:
        with (
            tc.tile_pool(name="const", bufs=1) as pconst,
            tc.tile_pool(name="zin", bufs=2) as pzin,
            tc.tile_pool(name="ep", bufs=6) as pep,
            tc.tile_pool(name="csb", bufs=2) as pcs,
            tc.tile_pool(name="outp", bufs=2) as pout,
            tc.tile_pool(name="psp", bufs=2, space="PSUM") as ppsum,
            tc.tile_pool(name="pcol", bufs=2, space="PSUM") as ppcol,
        ):
            bias_m2 = pconst.tile([128, 1], F32)
            nc.gpsimd.memset(bias_m2, -2.0)
            # 32-wide all-ones stationary: col-sum matmuls write 32
            # identical partition rows, keeping the PSUM bank contiguously
            # initialized for the one-instr DVE evacuation
            ones = pconst.tile([128, 32], F16)
            nc.gpsimd.memset(ones, 1.0)

            for _rep in range(reps):
                zq = pzin.tile([128, 2, COLS_LOC], F8, name="zq")
                for j in range(4):
                    c0, c1 = j * 1280, (j + 1) * 1280
                    nc.sync.dma_start(zq[:, :, c0:c1], zq_dram[:, :, c0:c1])

                rs = pout.tile([128, N_ACT], F32, name="rs")
                rs16 = pout.tile([128, N_DVE, 3], F16, name="rs16")

                ets = {}   # T -> (et tile f16-view for col matmuls)

                def mains(T):
                    hr, g, m = TRIPLES[T]
                    st = zq[:, :, 512 * hr + 128 * m : 512 * hr + 128 * m + 128]
                    ps = ppsum.tile([128, 3, HB], F32, name="ps")
                    for j in range(3):
                        d = 3 * g + j
                        c0 = 512 * (hr + d)
                        nc.tensor.matmul(
                            ps[:, j, :],
                            st,
                            zq[:, :, c0 : c0 + HB],
                            start=True,
                            stop=True,
                            perf_mode=DR,
                        )
                    return ps

                def consume(T, ps):
                    hr, g, m = TRIPLES[T]
                    if T in DVE_SET:
                        et = pep.tile([128, 3, HB], I16, name="et")
                        nc.vector.tensor_scalar(
                            et.rearrange("p a b -> p (a b)"),
                            ps.rearrange("p a b -> p (a b)"),
                            float(B_DVE),
                            None,
                            ADD,
                        )
                        etv = et[:, :, :].bitcast(F16)
                        with nc.allow_low_precision("f16 row-sum partials"):
                            nc.vector.tensor_reduce(
                                rs16[:, DVE_SLOT[T], :],
                                etv,
                                AXX,
                                ADD,
                            )
                        return etv
                    et = pep.tile([128, 3, HB], F16, name="et")
                    nc.scalar.activation(
                        et.rearrange("p a b -> p (a b)"),
                        ps.rearrange("p a b -> p (a b)"),
                        EXP,
                        bias=bias_m2,
                        scale=float(2.0 / ALPHA),
                        accum_out=rs[:, ACT_SLOT[T] : ACT_SLOT[T] + 1],
                    )
                    return et[:, :, :]

                def cols_group(hr, g):
                    # col sums for blocks (hr, d=3g+j), d-major so each
                    # PSUM accumulation group closes before the next opens
                    # in the same bank
                    cp = ppcol.tile([128, HB], F32, name="cp")
                    T0 = 12 * hr + 4 * g
                    joff = 1 if g == 0 else 0  # pack live d's from partition 0
                    live = [j for j in range(3) if 1 <= 3 * g + j <= 7]
                    for j in live:
                        p = 32 * (j - joff)  # matmul out base: 0/32/64
                        for m in range(4):
                            nc.tensor.matmul(
                                cp[p : p + 32, :],
                                ones,
                                ets[T0 + m][:, j, :],
                                start=(m == 0),
                                stop=(m == 3),
                            )
                    for m in range(4):
                        del ets[T0 + m]
                    # evacuate the initialized partition span in one DVE
                    # instr (cost is free-size-based), DMA one row per d
                    p1 = 32 * len(live)
                    csb = pcs.tile([96, HB], F32, name="csb")
                    nc.vector.tensor_scalar(
                        csb[0:p1, :],
                        cp[0:p1, :],
                        1.0,
                        None,
                        mybir.AluOpType.mult,
                    )
                    for j in live:
                        nc.gpsimd.dma_start(
                            cs_dram[hr, g, j, :],
                            csb[32 * (j - joff) : 32 * (j - joff) + 1, :],
                        )

                for T in range(24):
                    ps = mains(T)
                    ets[T] = consume(T, ps)
                    if T >= 2 and TRIPLES[T - 2][2] == 3:
                        hr2, g2, _ = TRIPLES[T - 2]
                        cols_group(hr2, g2)
                cols_group(1, 2)

                nc.gpsimd.dma_start(rs_dram[:], rs[:, :])
                nc.gpsimd.dma_start(
                    rs16_dram[:], rs16.rearrange("p a b -> p (a b)")
                )

    nc.compile()
    _cache[reps] = nc
    return nc


def _prep_inputs(z: np.ndarray):
    """z: [8192, 256] fp32.  Returns (zn fp64, znq fp64-of-fp8, in_maps)."""
    z64 = z.astype(np.float64)
    nrm = np.sqrt((z64 * z64).sum(1))
    zn = z64 / np.maximum(nrm, EPS)[:, None]
    znq8 = (zn * SQA).astype(np.float32).astype(NP_F8)
    znq = znq8.astype(np.float64) / SQA
    in_maps = []
    for c in range(N_CORES):
        rolled = np.roll(znq8, -ROWS_PER_CORE * c, axis=0)[:COLS_LOC]  # [5120, 256]
        # zq[p, i, r] = znq8[(r + 1024c) % 8192, 128*i + p]
        zt = np.ascontiguousarray(rolled.reshape(COLS_LOC, 2, 128).transpose(2, 1, 0))
        in_maps.append({"zq": zt})
    return zn, znq, in_maps


def _run_device(in_maps, trace: bool = False):
    nc = _build()
    res = bass_utils.run_bass_kernel_spmd(
        nc, in_maps, core_ids=list(range(N_CORES)), trace=trace
    )
    S = np.zeros(TWO_N, np.float64)
    ar = np.arange(128)
    ac = np.arange(HB)
    for c in range(N_CORES):
        rs = np.asarray(res.results[c]["rs"]).astype(np.float64)  # [128, N_ACT]
        rs16 = np.asarray(res.results[c]["rs16"]).astype(np.float64)
        cs = np.asarray(res.results[c]["cs"]).astype(np.float64)  # [2,3,3,HB]
        base = ROWS_PER_CORE * c
        for T, (hr, g, m) in enumerate(TRIPLES):
            rows = (base + 512 * hr + 128 * m + ar) % TWO_N
            if T in DVE_SET:
                S[rows] += rs16[:, DVE_SLOT[T], :].sum(-1)
            else:
                S[rows] += rs[:, ACT_SLOT[T]]
        for hr in range(2):
            for g in range(3):
                for j in range(3):
                    d = 3 * g + j
                    if d == 0 or d == 8:
                        continue
                    idx = (base + 512 * (hr + d) + ac) % TWO_N
                    S[idx] += cs[hr, g, j]
    return S, res.exec_time_ns


def _finalize(zn: np.ndarray, znq: np.ndarray, S: np.ndarray):
    selfsim_q = (znq * znq).sum(1)
    masked = S - np.exp(2.0 * selfsim_q - 2.0)
    lse = 2.0 + np.log(masked)
    pos = 2.0 * (zn * np.roll(zn, -N, axis=0)).sum(1)
    term = lse - pos
    loss_i = term[:N].sum() / N
    loss_j = term[N:].sum() / N
    return np.float32(loss_i), np.float32(loss_j)


def kernel(**inputs) -> np.ndarray:
    z_i = np.asarray(inputs["z_i"], dtype=np.float32)
    z_j = np.asarray(inputs["z_j"], dtype=np.float32)
    z = np.concatenate([z_i, z_j], axis=0)
    zn, znq, in_maps = _prep_inputs(z)
    S, _ = _run_device(in_maps, trace=False)
    return _finalize(zn, znq, S)
